# revision 27
# baseline (speedup 1.0000x reference)
"""GWPooling2D forward on 8 Trainium2 NeuronCores.

y[b, c, o] = sum_k m[c, o, k] * x[b, k]   (k = 400 input pixels, o = 256)

The pooling map m depends only on the small `signal` parameter and is
computed on host exactly as in the reference. It decomposes as

    m[c] = P0 + E[c]

where P0 (256 x 400) is the signal-independent resampling map (expm(0)=I
pushed through the same crop/roll/FFT pipeline) shared by all 16 channels,
and E[c] is the small per-channel correction (||E|| ~ 0.17 ||m||).

Device work per core (1024-batch shard, data parallel across 8 cores):
  yP = x_bf16 @ P0_bf16^T            (bf16 matmuls, 256 cols)
  yE = x_fp8  @ (E * s)_fp8^T        (fp8e4m3 DoubleRow matmuls, 4096 cols,
                                      K=400 in 2 packed chunks of 2x128/2x72)
yE is written back as fp8 (it is ~17% of y, so fp8 noise is ~0.6% of y),
yP as bf16; the host computes y = yP + yE/s. The single scale s keeps both
the quantized E and the yE PSUM values inside fp8e4m3 range (+-240).
"""

import numpy as np
import scipy.linalg

import concourse.bass as bass
import concourse.bacc as bacc
import concourse.mybir as mybir
import concourse.tile as tile
from concourse.bass_utils import run_bass_kernel_spmd
import ml_dtypes

C = 16
P = (24, 24)
NI = (20, 20)
NO = (16, 16)
B = 8192
NCORES = 8
BS = B // NCORES              # 1024 batch rows per core
K = NI[0] * NI[1]             # 400 contraction
O = NO[0] * NO[1]             # 256 output positions per channel
CO = C * O                    # 4096 (c,o) output columns
BT = 128                      # batch tile (PSUM partitions)
OT = 512                      # output-feature tile (PSUM free dim)
K0 = 256                      # DoubleRow chunk 0 (2 x 128 partitions)
K1 = K - K0                   # 144 = 2 x 72 partitions

F8 = ml_dtypes.float8_e4m3
BF16 = ml_dtypes.bfloat16


# ---------------------------------------------------------------- host map ---

def _hann(n):
    return 0.5 * (1.0 - np.cos(2.0 * np.pi * np.arange(n) / n))


def _signal_to_spectrum(signal):
    n0, n1 = signal.shape[-2], signal.shape[-1]
    window = _hann(n0)[:, None] * _hann(n1)[None, :]
    rx = np.arange((-n0) // 2 + 1, n0 // 2 + 1)[:, None]
    ry = np.arange((-n1) // 2 + 1, n1 // 2 + 1)[None, :]
    r = (1 + rx * rx + ry * ry).astype(np.float64)
    wf = np.roll(np.fft.fft2(signal), (n0 // 2, n1 // 2), (-2, -1)) / r / 5.0
    wt = np.fft.ifft2(np.roll(wf, (-(n0 // 2), -(n1 // 2)), (-2, -1))) * window
    return np.roll(np.fft.fft2(wt), (n0 // 2, n1 // 2), (-2, -1))


def _gw2d_algebra(w):
    p0, p1 = w.shape[-2], w.shape[-1]
    pad = [(0, 0)] * (w.ndim - 2) + [(p1 // 2, p1 // 2), (p0 // 2, p0 // 2)]
    wp = np.pad(w, pad)
    ia = np.arange(p0)[:, None] + np.arange(p0)[None, :]
    jb = np.arange(p1)[:, None] + np.arange(p1)[None, :]
    ws = wp[..., ia[:, None, :, None], jb[None, :, None, :]]
    ws = ws[..., ::-1, ::-1, :, :]
    kx = np.arange((-p0) // 2 + 1, p0 // 2 + 1)[:, None]
    ky = np.arange((-p1) // 2 + 1, p1 // 2 + 1)[None, :]
    return -1j * (ws[..., 0, :, :, :, :] * kx + ws[..., 1, :, :, :, :] * ky)


def _transform_to_map(t):
    p0, p1 = t.shape[-2], t.shape[-1]
    di = (p0 - NI[0], p1 - NI[1])
    do = (p0 - NO[0], p1 - NO[1])
    x = t[..., do[0] // 2 + 1:(-do[0]) // 2 + 1, do[1] // 2 + 1:(-do[1]) // 2 + 1,
          di[0] // 2 + 1:(-di[0]) // 2 + 1, di[1] // 2 + 1:(-di[1]) // 2 + 1]
    x = np.roll(x, (NO[0] // 2 + 1, NO[1] // 2 + 1, NI[0] // 2 + 1, NI[1] // 2 + 1),
                (-4, -3, -2, -1))
    return np.fft.fft2(np.fft.ifft2(x, axes=(-2, -1)), axes=(-4, -3)).real


def compute_mf(signal):
    """signal (C,2,24,24) -> pooling matrix (CO=4096, K=400) float32."""
    spectrum = _signal_to_spectrum(signal.astype(np.float64))
    p0, p1 = spectrum.shape[-2], spectrum.shape[-1]
    a = _gw2d_algebra(spectrum)
    n = p0 * p1
    mat = a.reshape(a.shape[:-4] + (n, n))
    t = np.stack([scipy.linalg.expm(mat[i]) for i in range(mat.shape[0])])
    t = t.reshape(t.shape[:-2] + (p0, p1, p0, p1))
    m = _transform_to_map(t)
    return m.reshape(CO, K).astype(np.float32)


_P0 = None


def compute_p0():
    """Signal-independent part of the map: expm(0)=I through the same
    crop/roll/FFT pipeline. (256, 400) float64."""
    global _P0
    if _P0 is None:
        t_id = np.eye(P[0] * P[1], dtype=np.complex128).reshape(
            1, P[0], P[1], P[0], P[1])
        _P0 = _transform_to_map(t_id).reshape(O, K)
    return _P0


# ------------------------------------------------------------ device kernel ---

_built = None


def _build():
    global _built
    if _built is not None:
        return _built
    nc = bacc.Bacc(dynamic_dma_scratch_size=16384)
    f32 = mybir.dt.float32
    bf16 = mybir.dt.bfloat16
    f8 = mybir.dt.float8e4
    DR = mybir.MatmulPerfMode.DoubleRow

    # x: fp8 DoubleRow packing, hi and lo-residual as separate tensors
    xh0_d = nc.declare_dram_parameter("xh0", (K0 // 2, 2, BS), f8, isOutput=False)
    xh1_d = nc.declare_dram_parameter("xh1", (K1 // 2, 2, BS), f8, isOutput=False)
    xl0_d = nc.declare_dram_parameter("xl0", (K0 // 2, 2, BS), f8, isOutput=False)
    xl1_d = nc.declare_dram_parameter("xl1", (K1 // 2, 2, BS), f8, isOutput=False)
    # P0 hi/lo/lo-shifted stack: cols [P1 | P2 | P3]
    pc0_d = nc.declare_dram_parameter("pc0", (K0 // 2, 2, 3 * O), f8, isOutput=False)
    pc1_d = nc.declare_dram_parameter("pc1", (K1 // 2, 2, 3 * O), f8, isOutput=False)
    e80_d = nc.declare_dram_parameter("e80", (K0 // 2, 2, CO), f8, isOutput=False)
    e81_d = nc.declare_dram_parameter("e81", (K1 // 2, 2, CO), f8, isOutput=False)
    outE_d = nc.declare_dram_parameter("outE", (BS, CO), f8, isOutput=True)
    outP_d = nc.declare_dram_parameter("outP", (BS, O), bf16, isOutput=True)

    NB = BS // BT                 # 8 batch tiles
    G = 4                         # co-tiles per staging/store/PSUM tile
    NEQ = 4
    EQ = CO // NEQ                # 1024 columns per E-load quarter

    with tile.TileContext(nc) as tc:
        with tc.tile_pool(name="inpool", bufs=1) as inpool, \
             tc.tile_pool(name="opool", bufs=8) as opool, \
             tc.tile_pool(name="pepool", bufs=4, space="PSUM") as pepool:
            e80q, e81q = [None] * NEQ, [None] * NEQ

            def load_equarter(q, ncols=EQ):
                t0 = inpool.tile([K0 // 2, 2, ncols], f8, tag=f"e80q{q}",
                                 name=f"e80q{q}")
                nc.sync.dma_start(t0[:], e80_d[:, :, q * EQ:q * EQ + ncols])
                e80q[q] = t0
                t1 = inpool.tile([K1 // 2, 2, ncols], f8, tag=f"e81q{q}",
                                 name=f"e81q{q}")
                nc.sync.dma_start(t1[:], e81_d[:, :, q * EQ:q * EQ + ncols])
                e81q[q] = t1

            # PE warmup: p-state ramp completes while loads land
            warm = inpool.tile([128, 2, 128], f8, name="warm")
            nc.vector.memset(warm[:], 0.0)
            wps = pepool.tile([BT, 2 * OT], f32, name="ps")
            for _ in range(64):
                nc.tensor.matmul(wps[:, :BT], warm[:], warm[:],
                                 start=True, stop=True, perf_mode=DR)

            xh0 = inpool.tile([K0 // 2, 2, BS], f8, name="xh0")
            nc.sync.dma_start(xh0[:], xh0_d[:])
            xh1 = inpool.tile([K1 // 2, 2, BS], f8, name="xh1")
            nc.sync.dma_start(xh1[:], xh1_d[:])
            pc0 = inpool.tile([K0 // 2, 2, 3 * O], f8, name="pc0")
            nc.sync.dma_start(pc0[:], pc0_d[:])
            pc1 = inpool.tile([K1 // 2, 2, 3 * O], f8, name="pc1")
            nc.sync.dma_start(pc1[:], pc1_d[:])
            xl0 = inpool.tile([K0 // 2, 2, BS], f8, name="xl0")
            nc.sync.dma_start(xl0[:], xl0_d[:])
            xl1 = inpool.tile([K1 // 2, 2, BS], f8, name="xl1")
            nc.sync.dma_start(xl1[:], xl1_d[:])
            load_equarter(0)
            load_equarter(1)
            load_equarter(2)
            load_equarter(3)

            # ---- P part first: overlaps the E-map load phase ----
            def p_group(pi, b, yps):
                # yP*s_p = x_hi@(P1+P2) + x_lo'@P3, all fp8 DoubleRow
                pp = pepool.tile([BT, 2 * OT], f32, name="ps")
                for r in range(3):
                    xa0 = xh0 if r < 2 else xl0
                    xa1 = xh1 if r < 2 else xl1
                    nc.tensor.matmul(
                        pp[:, :O],
                        xa0[:, :, b * BT:(b + 1) * BT],
                        pc0[:, :, r * O:(r + 1) * O],
                        start=(r == 0), stop=False, perf_mode=DR,
                    )
                    nc.tensor.matmul(
                        pp[:, :O],
                        xa1[:, :, b * BT:(b + 1) * BT],
                        pc1[:, :, r * O:(r + 1) * O],
                        start=False, stop=(r == 2), perf_mode=DR,
                    )
                nc.any.tensor_copy(yps[:, b % 4, :], pp[:, :O])

            def e_group(i, b, cp):
                # one staging tile = 4 co-tiles = 2 double-bank PSUM tiles
                st = opool.tile([BT, G * OT], f8, name="st")
                for h in range(2):
                    ps = pepool.tile([BT, 2 * OT], f32, name="ps")
                    for j2 in range(2):
                        co = cp * G + h * 2 + j2
                        col = co * OT
                        q = col // EQ
                        cof = col - q * EQ
                        nc.tensor.matmul(
                            ps[:, j2 * OT:(j2 + 1) * OT],
                            xh0[:, :, b * BT:(b + 1) * BT],
                            e80q[q][:, :, cof:cof + OT],
                            start=True, stop=False, perf_mode=DR,
                        )
                        nc.tensor.matmul(
                            ps[:, j2 * OT:(j2 + 1) * OT],
                            xh1[:, :, b * BT:(b + 1) * BT],
                            e81q[q][:, :, cof:cof + OT],
                            start=False, stop=True, perf_mode=DR,
                        )
                    nc.any.tensor_copy(
                        st[:, h * 2 * OT:(h + 1) * 2 * OT], ps[:])
                nc.sync.dma_start(
                    outE_d[b * BT:(b + 1) * BT, cp * G * OT:(cp + 1) * G * OT],
                    st[:])

            yps0 = opool.tile([BT, 4, O], bf16, tag="yps0", name="yps0")
            yps1 = opool.tile([BT, 4, O], bf16, tag="yps1", name="yps1")

            for pi in range(NB):
                p_group(pi, pi, yps0 if pi < 4 else yps1)
                if pi == 3:
                    nc.sync.dma_start(
                        outP_d[:4 * BT].rearrange("(j p) o -> p j o", p=BT),
                        yps0[:])
                elif pi == 7:
                    nc.sync.dma_start(
                        outP_d[4 * BT:].rearrange("(j p) o -> p j o", p=BT),
                        yps1[:])

            e_list = [(b, 0) for b in range(NB)] + [(b, 1) for b in range(NB)]
            for i, (b, cp) in enumerate(e_list):
                e_group(i, b, cp)
    nc.compile()
    _built = nc
    return nc


SX = 64.0


def _prep_host(x, signal):
    """Host-side factorization + quantization. Returns per-core input maps
    and the dequantization scales (s_e, s_p)."""
    mf = compute_mf(np.asarray(signal))                     # (4096, 400)
    p0 = compute_p0()                                       # (256, 400) f64
    e = mf.astype(np.float64).reshape(C, O, K) - p0[None]
    ef = e.reshape(CO, K)

    # E path scale: keeps E*s inside fp8 range and (with 8-sigma slack for
    # x ~ N(0,1)) the yE accumulator inside +-240 at the fp8 store
    row_norm = np.sqrt((ef * ef).sum(axis=1)).max()
    s_e = min(200.0 / np.abs(ef).max(), 200.0 / (8.0 * row_norm))
    e8 = (ef * s_e).astype(np.float32).astype(F8)           # (4096, 400)
    e8c0 = np.ascontiguousarray(
        e8[:, :K0].reshape(CO, 2, K0 // 2).transpose(2, 1, 0))   # (128,2,4096)
    e8c1 = np.ascontiguousarray(
        e8[:, K0:].reshape(CO, 2, K1 // 2).transpose(2, 1, 0))   # (72,2,4096)

    # P path: hi/lo fp8 split of P0 (and of x), shared x_hi with the E path
    s_p = 200.0 / np.abs(p0).max()
    p1 = (p0 * s_p).astype(np.float32).astype(F8)
    p2 = (p0 * s_p - p1.astype(np.float64)).astype(np.float32).astype(F8)
    p3 = ((p1.astype(np.float32) + p2.astype(np.float32)) / SX).astype(F8)
    p123 = np.concatenate(
        [p1.astype(np.float32), p2.astype(np.float32), p3.astype(np.float32)],
        axis=0)                                             # (3*O, K)
    pc0 = np.ascontiguousarray(
        p123.astype(F8)[:, :K0].reshape(3 * O, 2, K0 // 2).transpose(2, 1, 0))
    pc1 = np.ascontiguousarray(
        p123.astype(F8)[:, K0:].reshape(3 * O, 2, K1 // 2).transpose(2, 1, 0))

    xT = np.asarray(x).reshape(B, K).T.astype(np.float32)   # (400, 8192)
    x_hi = xT.astype(F8)
    x_lo = ((xT - x_hi.astype(np.float32)) * SX).astype(F8)

    def pack_x(xq, k_lo, k_hi, kil):
        return xq[k_lo:k_hi].reshape(2, kil, B).transpose(1, 0, 2)

    xh0 = pack_x(x_hi, 0, K0, K0 // 2)
    xh1 = pack_x(x_hi, K0, K, K1 // 2)
    xl0 = pack_x(x_lo, 0, K0, K0 // 2)
    xl1 = pack_x(x_lo, K0, K, K1 // 2)

    in_maps = []
    for i in range(NCORES):
        bs = slice(i * BS, (i + 1) * BS)
        in_maps.append({
            "xh0": np.ascontiguousarray(xh0[:, :, bs]),
            "xh1": np.ascontiguousarray(xh1[:, :, bs]),
            "xl0": np.ascontiguousarray(xl0[:, :, bs]),
            "xl1": np.ascontiguousarray(xl1[:, :, bs]),
            "pc0": pc0,
            "pc1": pc1,
            "e80": e8c0,
            "e81": e8c1,
        })
    return in_maps, s_e, s_p


def _run(x, signal, **spmd_kwargs):
    nc = _build()
    in_maps, s_e, s_p = _prep_host(x, signal)
    res = run_bass_kernel_spmd(nc, in_maps, list(range(NCORES)), **spmd_kwargs)
    parts = []
    for r in res.results:
        yE = r["outE"].astype(np.float32).reshape(BS, C, O) / s_e
        yP = r["outP"].astype(np.float32) / s_p
        parts.append(yE + yP[:, None, :])
    y = np.concatenate(parts, axis=0)
    return y.reshape(B, C, NO[0], NO[1]), res


def kernel(x, signal):
    y, _ = _run(x, signal)
    return y
